# revision 30
# baseline (speedup 1.0000x reference)
"""Trainium2 kernel for nn_KermtAutoregressive (T=2048 autoregressive MLP stack).

Structure: the only sequential dependency is the scalar prev_rf, entering the
beta head as gelu(base_t + p * w_rf) with ||w_rf|| ~ 0.02, so mu_t(p)/phi_t(p)
are nearly-linear in p.  The heavy MLP stacks are evaluated batched over t at
two nodes p in {+1, -1} (node -1 only for the first WIN=256 global steps: rf
saturates to exactly 1.0 by t=14, validated), then a cheap host-side
fixed-point (cumprod) resolves the recurrence.  Device work is data-parallel
over t across 8 NeuronCores (256 own rows + 32 transient rows per core).

Device layout: activations are feature-major ([feat partition, rows free]),
weights bf16 as [K,M] stationary tiles streamed from a host-packed DRAM image,
fp32 PSUM accumulation.  LayerNorm stats (mean / E[x^2]) via ones-vector
matmuls on the PE, broadcast back through a K=1 matmul, applied with two DVE
tensor-tensor ops.  Biases ride free in ScalarE activation bias operands;
LN gain/bias of stack-final blocks are folded into downstream weights on host.
"""

import numpy as np
import ml_dtypes

M = 1024
T = 2048
NS = 5
DE = 64
NB = 3
NCORES = 8
RPC = T // NCORES            # own rows per core (256)
EPS = 1e-5
NI = 6                       # host fixed-point iterations

BF = ml_dtypes.bfloat16

TRACE = False                # test.py sets kernel.TRACE = True for profiling
LAST_RESULTS = None          # stashed BassKernelResults for test.py


def _bf(x):
    return np.ascontiguousarray(np.asarray(x, np.float32).astype(BF))


def _tile_mat(w):
    """[1024,1024] -> [128, 8*8*128] bf16 row-image: row p holds, for kt, of,
    W[kt*128+p, of*128:(of+1)*128] at free offset (kt*8+of)*128."""
    w = np.asarray(w, np.float32)
    return _bf(w.reshape(8, 128, 8, 128).transpose(1, 0, 2, 3).reshape(128, 8192))


class _Layout:
    """Free-dim element offsets inside the misc block of wpack (bf16)."""

    def __init__(self, cfg):
        self.cfg = cfg
        off = 0

        def take(n):
            nonlocal off
            o = off
            off += n
            return o

        self.peff = take(M)             # [5, 1024] on partitions 0:5
        self.descw = take(DE)           # [6, 64] on partitions 0:6
        self.ident = take(128)          # [128, 128]
        self.ones_col = take(1)         # [128, 1] value 1/1024
        self.ones_row = take(128)       # [1, 128] value 1.0
        self.wde = take(M)              # [64, 8*128]: W_de[64,1024] natural
        self.head = take(16)            # [128, 8*2]: head kt tile at +kt*2
        if cfg["b2sp"]:
            self.b2sp = take(3 * M)     # [1, 3*1024] partition 0
        if cfg["b2bh"]:
            self.b2bh = take(3 * M)
        self.misc_len = off
        self.mats = off                 # 13 big matrices follow
        self.total = off + 13 * 8192

        # smalls (fp32 [128, S]) column offsets
        s = 0

        def stake(n):
            nonlocal s
            o = s
            s += n
            return o

        self.s_spb = stake(8)           # sp_proj_b tiles
        self.s_descb = stake(1)         # desc_b on partitions 0:64
        self.s_b1sp = stake(24)         # 3 blocks x 8
        self.s_b1bh = stake(24)
        self.s_zb = stake(16)           # z bias: [:, 0:8]=node +1, [:, 8:16]=node -1
        if cfg["gbsp"]:
            self.s_gsp = stake(16)      # blocks 0,1: g tiles
            self.s_bsp = stake(16)
        if cfg["gbbh"]:
            self.s_gbh = stake(16)
            self.s_bbh = stake(16)
        self.smalls_len = s


_BUILD_CACHE = {}


def _build_program(cfg):
    """Build (nc, layout) for a given cfg. cfg keys:
    ncol: device column count (own rows + transient rows)
    nt:   transient columns (ncol = RPC + nt)
    b2sp/b2bh: emit fc2-bias row matmuls; gbsp/gbbh: emit inner-LN gain/bias.
    """
    key = tuple(sorted(cfg.items()))
    if key in _BUILD_CACHE:
        return _BUILD_CACHE[key]

    import concourse.bass as bass
    import concourse.bacc as bacc
    import concourse.tile as tile
    import concourse.mybir as mybir
    from contextlib import ExitStack

    lay = _Layout(cfg)
    NCOL = cfg["ncol"]
    NT = cfg["nt"]
    F32 = mybir.dt.float32
    BF16 = mybir.dt.bfloat16
    AF = mybir.ActivationFunctionType

    nc = bacc.Bacc("TRN2", target_bir_lowering=False)

    wpack = nc.dram_tensor("wpack", [128, lay.total], BF16, kind="ExternalInput")
    smalls = nc.dram_tensor("smalls", [128, lay.smalls_len], F32, kind="ExternalInput")
    ratt = nc.dram_tensor("ratt", [NS, NCOL], BF16, kind="ExternalInput")
    dest_ = nc.dram_tensor("dest", [6, NCOL], BF16, kind="ExternalInput")
    out = nc.dram_tensor("out", [2, NCOL], F32, kind="ExternalOutput")
    dbg_stage = cfg.get("dbg")
    dbg = None
    if dbg_stage:
        dbg = nc.dram_tensor("dbg", [128, 8, NCOL], BF16, kind="ExternalOutput")

    with tile.TileContext(nc) as tc, ExitStack() as ctx:
        const = ctx.enter_context(tc.tile_pool(name="const", bufs=1))
        wpool = ctx.enter_context(tc.tile_pool(name="wpool", bufs=6))
        xpool = ctx.enter_context(tc.tile_pool(name="xpool", bufs=2))
        hpool = ctx.enter_context(tc.tile_pool(name="hpool", bufs=3))
        spool = ctx.enter_context(tc.tile_pool(name="spool", bufs=4))
        bpool = ctx.enter_context(tc.tile_pool(name="bpool", bufs=4))
        tpool = ctx.enter_context(tc.tile_pool(name="tpool", bufs=12))
        rpool = ctx.enter_context(tc.tile_pool(name="rpool", bufs=10))
        zpool = ctx.enter_context(tc.tile_pool(name="zpool", bufs=1))
        pmm = ctx.enter_context(tc.tile_pool(name="pmm", bufs=4, space="PSUM"))
        prow = ctx.enter_context(tc.tile_pool(name="prow", bufs=2, space="PSUM"))
        pbc = ctx.enter_context(tc.tile_pool(name="pbc", bufs=1, space="PSUM"))

        # ---- constants / small inputs ----
        misc = const.tile([128, lay.misc_len], BF16, tag="misc")
        nc.sync.dma_start(out=misc, in_=wpack[:, 0:lay.misc_len])
        sm = const.tile([128, lay.smalls_len], F32, tag="sm")
        nc.sync.dma_start(out=sm, in_=smalls[:, :])
        rt = const.tile([NS, NCOL], BF16, tag="rt")
        nc.sync.dma_start(out=rt, in_=ratt[:, :])
        dt_ = const.tile([6, NCOL], BF16, tag="dt")
        nc.sync.dma_start(out=dt_, in_=dest_[:, :])

        ident = misc[:, lay.ident:lay.ident + 128]
        ones_col = misc[:, lay.ones_col:lay.ones_col + 1]
        ones_row = misc[0:1, lay.ones_row:lay.ones_row + 128]
        eps_t = const.tile([1, 1], F32, tag="eps")
        nc.vector.memset(eps_t, EPS)
        # ACT instructions encode a single sync-wait; touch the sm DMA once on
        # ScalarE so later ACT bias reads never add a second (DMA) wait.
        warm = const.tile([1, 1], F32, tag="warm")
        nc.scalar.copy(warm, sm[0:1, 0:1])

        def load_mat(i):
            w = wpool.tile([128, 8192], BF16, tag="wmat")
            o = lay.mats + i * 8192
            nc.sync.dma_start(out=w, in_=wpack[:, o:o + 8192])
            return w

        def wt(w, kt, of):
            o = (kt * 8 + of) * 128
            return w[:, o:o + 128]

        def resblock(x, w1, w2, b1_col0, b2_off, gb, n_im, blk=""):
            """x: bf16 tile [128, 8, NCOL] -> returns next x (post-LN)."""
            # fc1 + gelu(+b1)
            h = hpool.tile([128, 8, NCOL], BF16, tag="h")
            for of in range(8):
                ph = pmm.tile([128, NCOL], F32, tag="pmm")
                for kt in range(8):
                    nc.tensor.matmul(ph, lhsT=wt(w1, kt, of), rhs=x[:, kt, :],
                                     start=(kt == 0), stop=(kt == 7))
                nc.scalar.activation(h[:, of, :], ph, AF.Gelu,
                                     bias=sm[:, b1_col0 + of:b1_col0 + of + 1],
                                     scale=1.0)
            dbg_dump(blk + "h", h)
            # fc2 (+b2 row) + residual; copy to sbuf; x^2; stats
            s = spool.tile([128, 8, NCOL], BF16, tag="s")
            pm = prow.tile([1, NCOL], F32, tag="prow")
            pe = prow.tile([1, NCOL], F32, tag="prow")
            for of in range(8):
                ps = pmm.tile([128, NCOL], F32, tag="pmm")
                for kt in range(8):
                    nc.tensor.matmul(ps, lhsT=wt(w2, kt, of), rhs=h[:, kt, :],
                                     start=(kt == 0), stop=False)
                if b2_off is not None:
                    nc.tensor.matmul(ps, lhsT=misc[0:1, b2_off + of * 128:
                                                    b2_off + of * 128 + 128],
                                     rhs=ones_n, start=False, stop=False)
                nc.tensor.matmul(ps, lhsT=ident, rhs=x[:, of, :],
                                 start=False, stop=True)
                nc.scalar.activation(s[:, of, :], ps, AF.Copy)
                x2 = tpool.tile([128, NCOL], BF16, tag="x2")
                nc.vector.tensor_mul(x2, s[:, of, :], s[:, of, :])
                nc.tensor.matmul(pm, lhsT=ones_col, rhs=s[:, of, :],
                                 start=(of == 0), stop=(of == 7))
                nc.tensor.matmul(pe, lhsT=ones_col, rhs=x2,
                                 start=(of == 0), stop=(of == 7))
            dbg_dump(blk + "s", s)
            # rows: m, inv = 1/sqrt(E - m^2 + eps)
            rowb = rpool.tile([1, 2, NCOL], BF16, tag="rowb")
            nc.vector.tensor_copy(rowb[:, 0, :], pm)
            e2s = rpool.tile([1, NCOL], F32, tag="rtmp")
            nc.vector.tensor_copy(e2s, pe)
            msq = rpool.tile([1, NCOL], F32, tag="rtmp")
            nc.vector.tensor_mul(msq, rowb[:, 0, :], rowb[:, 0, :])
            var = rpool.tile([1, NCOL], F32, tag="rtmp")
            nc.vector.tensor_sub(var, e2s, msq)
            sd = rpool.tile([1, NCOL], F32, tag="rtmp")
            nc.scalar.activation(sd, var, AF.Sqrt, bias=eps_t, scale=1.0)
            with nc.allow_low_precision(reason="inv feeds bf16 multiply"):
                nc.vector.reciprocal(rowb[:, 1, :], sd)
            # broadcast [m | inv] to 128 partitions
            pb = pbc.tile([128, 2 * NCOL], F32, tag="pbc")
            flat = rowb.rearrange("p a b -> p (a b)")
            for c0 in range(0, 2 * NCOL, 512):
                c1 = min(c0 + 512, 2 * NCOL)
                nc.tensor.matmul(pb[:, c0:c1], lhsT=ones_row,
                                 rhs=flat[:, c0:c1], start=True, stop=True)
            bsb = bpool.tile([128, 2 * NCOL], BF16, tag="bsb")
            nc.scalar.activation(bsb, pb, AF.Copy)
            # apply (s - m) * inv (+ optional g,b)
            xn = xpool.tile([128, 8, NCOL], BF16, tag="x")
            for of in range(8):
                tt = tpool.tile([128, NCOL], BF16, tag="x2")
                nc.vector.tensor_sub(tt, s[:, of, :], bsb[:, 0:NCOL])
                nc.vector.tensor_mul(xn[:, of, :], tt, bsb[:, NCOL:2 * NCOL])
                if gb is not None:
                    g_c, b_c = gb
                    nc.scalar.activation(
                        xn[:, of, :], xn[:, of, :], AF.Identity,
                        bias=sm[:, b_c + n_im * 8 + of:b_c + n_im * 8 + of + 1],
                        scale=sm[:, g_c + n_im * 8 + of:g_c + n_im * 8 + of + 1])
            return xn

        ones_n = None
        if cfg["b2sp"] or cfg["b2bh"]:
            ones_n = const.tile([1, NCOL], BF16, tag="ones_n")
            nc.vector.memset(ones_n, 1.0)

        def dbg_dump(name, t):
            if dbg_stage == name:
                nc.sync.dma_start(out=dbg[:, :, :], in_=t)

        # ---- solvent projection: sp_pre = gelu(P_eff.T-form @ ratiosT + b) ----
        x = xpool.tile([128, 8, NCOL], BF16, tag="x")
        for of in range(8):
            pp = pmm.tile([128, NCOL], F32, tag="pmm")
            nc.tensor.matmul(pp, lhsT=misc[0:NS, lay.peff + of * 128:
                                           lay.peff + of * 128 + 128],
                             rhs=rt, start=True, stop=True)
            nc.scalar.activation(x[:, of, :], pp, AF.Gelu,
                                 bias=sm[:, lay.s_spb + of:lay.s_spb + of + 1],
                                 scale=1.0)
        dbg_dump("sppre", x)

        # ---- sp residual blocks ----
        for i in range(NB):
            w1 = load_mat(2 * i)
            w2 = load_mat(2 * i + 1)
            b2o = (lay.b2sp + i * M) if cfg["b2sp"] else None
            gb = ((lay.s_gsp, lay.s_bsp) if (cfg["gbsp"] and i < 2) else None)
            x = resblock(x, w1, w2, lay.s_b1sp + i * 8, b2o, gb, i, blk=f"sp{i + 1}")
            dbg_dump(f"sp{i + 1}", x)

        # ---- desc embedding ----
        pd = prow.tile([DE, NCOL], F32, tag="prow")
        nc.tensor.matmul(pd, lhsT=misc[0:6, lay.descw:lay.descw + DE],
                         rhs=dt_, start=True, stop=True)
        demb = const.tile([DE, NCOL], BF16, tag="demb")
        nc.scalar.activation(demb, pd, AF.Gelu,
                             bias=sm[0:DE, lay.s_descb:lay.s_descb + 1], scale=1.0)

        # ---- base = W_sp' @ sp3 + W_de @ demb (psum), z = gelu(base + zbias_k) ----
        wsp = load_mat(6)
        z = zpool.tile([128, 8, NCOL], BF16, tag="z")
        for of in range(8):
            pbase = pmm.tile([128, NCOL], F32, tag="pmm")
            for kt in range(8):
                nc.tensor.matmul(pbase, lhsT=wt(wsp, kt, of), rhs=x[:, kt, :],
                                 start=(kt == 0), stop=False)
            nc.tensor.matmul(pbase, lhsT=misc[0:DE, lay.wde + of * 128:
                                              lay.wde + of * 128 + 128],
                             rhs=demb, start=False, stop=True)
            # own rows at node +1, transient rows at node -1
            nc.scalar.activation(z[:, of, 0:RPC], pbase[:, 0:RPC], AF.Gelu,
                                 bias=sm[:, lay.s_zb + of:lay.s_zb + of + 1],
                                 scale=1.0)
            nc.scalar.activation(z[:, of, RPC:NCOL], pbase[:, RPC:NCOL], AF.Gelu,
                                 bias=sm[:, lay.s_zb + 8 + of:lay.s_zb + 8 + of + 1],
                                 scale=1.0)

        # ---- beta head residual blocks ----
        x = z
        dbg_dump("z", x)
        for i in range(NB):
            w1 = load_mat(7 + 2 * i)
            w2 = load_mat(8 + 2 * i)
            b2o = (lay.b2bh + i * M) if cfg["b2bh"] else None
            gb = ((lay.s_gbh, lay.s_bbh) if (cfg["gbbh"] and i < 2) else None)
            x = resblock(x, w1, w2, lay.s_b1bh + i * 8, b2o, gb, i, blk=f"bh{i + 1}")
            dbg_dump(f"bh{i + 1}", x)

        # ---- head: [2, NCOL] logits ----
        po = prow.tile([2, NCOL], F32, tag="prow")
        for kt in range(8):
            nc.tensor.matmul(po, lhsT=misc[:, lay.head + kt * 2:lay.head + kt * 2 + 2],
                             rhs=x[:, kt, :], start=(kt == 0), stop=(kt == 7))
        osb = const.tile([2, NCOL], F32, tag="osb")
        nc.scalar.activation(osb, po, AF.Copy)
        nc.sync.dma_start(out=out[:, :], in_=osb)

    nc.compile()
    _BUILD_CACHE[key] = (nc, lay)
    return nc, lay


def kernel(**inputs):
    global LAST_RESULTS
    f = lambda k: np.asarray(inputs[k], np.float32)
    solv, desc = f("solvent_seq"), f("desc_seq")
    molv, sv = f("mol_vec"), f("solvent_vecs")
    bm = np.asarray(inputs["boundary_mask"]).astype(bool)

    sp_ln_g, sp_ln_b = f("sp_ln_g"), f("sp_ln_b")
    bh_ln_g, bh_ln_b = f("bh_ln_g"), f("bh_ln_b")
    sp_fc2_b, bh_fc2_b = f("sp_fc2_b"), f("bh_fc2_b")

    any_bound = bool(bm.any())
    nt = RPC if any_bound else 32          # transient rows per core
    win = nt * NCORES                      # global transient window
    cfg = {
        "ncol": RPC + nt,
        "nt": nt,
        "b2sp": not np.allclose(sp_fc2_b, 0.0),
        "b2bh": not np.allclose(bh_fc2_b, 0.0),
        "gbsp": not (np.allclose(sp_ln_g[:2], 1.0) and np.allclose(sp_ln_b[:2], 0.0)),
        "gbbh": not (np.allclose(bh_ln_g[:2], 1.0) and np.allclose(bh_ln_b[:2], 0.0)),
    }
    NCOL = cfg["ncol"]

    # ---------- host precompute / weight folding ----------
    Wp = f("sp_proj_w").reshape(NS, M, M)
    P_eff = np.stack([sv[s] @ Wp[s] for s in range(NS)]).astype(np.float32)  # [5, M]

    bh_proj_w = f("bh_proj_w")
    W_mol, W_sp = bh_proj_w[:M], bh_proj_w[M:2 * M]
    W_de, w_rf = bh_proj_w[2 * M:2 * M + DE], bh_proj_w[2 * M + DE]
    mol_const = molv @ W_mol

    # fold sp final LN (block 2): sp3 = g*n + b -> n @ (g*W_sp), b@W_sp into bias
    W_sp_f = sp_ln_g[2][:, None] * W_sp
    zb_extra = sp_ln_b[2] @ W_sp
    zb_core = mol_const + f("bh_proj_b") + zb_extra
    zb_p1 = (zb_core + 1.0 * w_rf).astype(np.float32)
    zb_m1 = (zb_core - 1.0 * w_rf).astype(np.float32)

    # fold bh final LN into head: logits = n @ (g*hw) + (b@hw + head_b)
    hw = np.stack([f("mu_w"), f("phi_w")], axis=1)       # [M, 2]
    hw_f = bh_ln_g[2][:, None] * hw
    logit_bias = bh_ln_b[2] @ hw + np.array([f("mu_b")[0], f("phi_b")[0]], np.float32)

    lay = _Layout(cfg)

    # ---------- wpack ----------
    wpack = np.zeros((128, lay.total), BF)
    mi = lay
    wpack[0:NS, mi.peff:mi.peff + M] = _bf(P_eff)
    wpack[0:6, mi.descw:mi.descw + DE] = _bf(f("desc_w"))
    wpack[:, mi.ident:mi.ident + 128] = _bf(np.eye(128, dtype=np.float32))
    wpack[:, mi.ones_col:mi.ones_col + 1] = _bf(np.full((128, 1), 1.0 / M))
    wpack[0:1, mi.ones_row:mi.ones_row + 128] = _bf(np.ones((1, 128)))
    wpack[0:DE, mi.wde:mi.wde + M] = _bf(W_de)           # [64, 1024] natural
    # head tiles: [128, 8*2]
    hh = hw_f.reshape(8, 128, 2).transpose(1, 0, 2).reshape(128, 16)
    wpack[:, mi.head:mi.head + 16] = _bf(hh)
    if cfg["b2sp"]:
        wpack[0:1, mi.b2sp:mi.b2sp + 3 * M] = _bf(sp_fc2_b.reshape(1, 3 * M))
    if cfg["b2bh"]:
        wpack[0:1, mi.b2bh:mi.b2bh + 3 * M] = _bf(bh_fc2_b.reshape(1, 3 * M))
    mats = [f("sp_fc1_w")[0], f("sp_fc2_w")[0],
            f("sp_fc1_w")[1], f("sp_fc2_w")[1],
            f("sp_fc1_w")[2], f("sp_fc2_w")[2],
            W_sp_f,
            f("bh_fc1_w")[0], f("bh_fc2_w")[0],
            f("bh_fc1_w")[1], f("bh_fc2_w")[1],
            f("bh_fc1_w")[2], f("bh_fc2_w")[2]]
    for i, w in enumerate(mats):
        o = lay.mats + i * 8192
        wpack[:, o:o + 8192] = _tile_mat(w)

    # ---------- smalls ----------
    sm = np.zeros((128, lay.smalls_len), np.float32)
    sm[:, lay.s_spb:lay.s_spb + 8] = f("sp_proj_b").reshape(8, 128).T
    sm[0:DE, lay.s_descb] = f("desc_b")
    sm[:, lay.s_b1sp:lay.s_b1sp + 24] = f("sp_fc1_b").reshape(3 * 8, 128).T
    sm[:, lay.s_b1bh:lay.s_b1bh + 24] = f("bh_fc1_b").reshape(3 * 8, 128).T
    sm[:, lay.s_zb:lay.s_zb + 8] = zb_p1.reshape(8, 128).T
    sm[:, lay.s_zb + 8:lay.s_zb + 16] = zb_m1.reshape(8, 128).T
    if cfg["gbsp"]:
        sm[:, lay.s_gsp:lay.s_gsp + 16] = sp_ln_g[:2].reshape(16, 128).T
        sm[:, lay.s_bsp:lay.s_bsp + 16] = sp_ln_b[:2].reshape(16, 128).T
    if cfg["gbbh"]:
        sm[:, lay.s_gbh:lay.s_gbh + 16] = bh_ln_g[:2].reshape(16, 128).T
        sm[:, lay.s_bbh:lay.s_bbh + 16] = bh_ln_b[:2].reshape(16, 128).T

    # ---------- per-core activations ----------
    ratT = _bf(solv.T)                                   # [5, T]
    desT = _bf(desc.T)                                   # [6, T]
    in_maps = []
    for c in range(NCORES):
        own = slice(c * RPC, (c + 1) * RPC)
        tr = slice(c * nt, (c + 1) * nt)
        in_maps.append({
            "wpack": wpack,
            "smalls": sm,
            "ratt": np.concatenate([ratT[:, own], ratT[:, tr]], axis=1),
            "dest": np.concatenate([desT[:, own], desT[:, tr]], axis=1),
        })

    # ---------- run on 8 NeuronCores ----------
    from concourse.bass_utils import run_bass_kernel_spmd
    nc, _ = _build_program(cfg)
    res = run_bass_kernel_spmd(nc, in_maps, core_ids=list(range(NCORES)),
                               trace=TRACE)
    LAST_RESULTS = res

    # ---------- host epilogue ----------
    L1 = np.empty((2, T), np.float32)                    # logits at p=+1
    L0 = np.empty((2, win), np.float32)                  # logits at p=-1 (window)
    for c in range(NCORES):
        o = res.results[c]["out"]
        L1[:, c * RPC:(c + 1) * RPC] = o[:, 0:RPC]
        L0[:, c * nt:(c + 1) * nt] = o[:, RPC:NCOL]
    L1 += logit_bias[:, None]
    L0 += logit_bias[:, None]

    def sigmoid(x):
        return (1.0 / (1.0 + np.exp(-x))).astype(np.float32)

    def softplus(x):
        return (np.log1p(np.exp(-np.abs(x))) + np.maximum(x, 0.0) + 2.0).astype(np.float32)

    mu1, phi1 = sigmoid(L1[0]), softplus(L1[1])
    mu0, phi0 = sigmoid(L0[0]), softplus(L0[1])

    # mu(p) linear in p on the window; == mu1 beyond it
    a_mu = np.concatenate([(mu1[:win] + mu0) * 0.5, mu1[win:]])
    b_mu = np.concatenate([(mu1[:win] - mu0) * 0.5, np.zeros(T - win, np.float32)])
    a_ph = np.concatenate([(phi1[:win] + phi0) * 0.5, phi1[win:]])
    b_ph = np.concatenate([(phi1[:win] - phi0) * 0.5, np.zeros(T - win, np.float32)])

    if not any_bound:
        p = np.ones(T, np.float32)
        p[0] = -1.0
        for _ in range(NI):
            mu = (a_mu + b_mu * p).astype(np.float32)
            with np.errstate(under="ignore"):
                rf = (1.0 - np.exp(np.cumsum(np.log1p(-mu), dtype=np.float32)))
            p = np.concatenate(([np.float32(-1.0)], rf[:-1].astype(np.float32)))
        mu = (a_mu + b_mu * p).astype(np.float32)
        phi = (a_ph + b_ph * p).astype(np.float32)
        with np.errstate(under="ignore"):
            rf = (1.0 - np.exp(np.cumsum(np.log1p(-mu), dtype=np.float32))).astype(np.float32)
    else:
        rf = np.empty(T, np.float32)
        mu = np.empty(T, np.float32)
        phi = np.empty(T, np.float32)
        prev = np.float32(-1.0)
        for t in range(T):
            mt = np.float32(a_mu[t] + b_mu[t] * prev)
            pt = np.float32(a_ph[t] + b_ph[t] * prev)
            r = mt if (bm[t] or prev < 0) else np.float32(prev + mt * (1.0 - prev))
            rf[t], mu[t], phi[t] = r, mt, pt
            prev = r

    return np.stack([rf, mu, phi]).astype(np.float32)


# revision 31
# speedup vs baseline: 1.0221x; 1.0221x over previous
"""Trainium2 kernel for nn_KermtAutoregressive (T=2048 autoregressive MLP stack).

Structure: the only sequential dependency is the scalar prev_rf, entering the
beta head as gelu(base_t + p * w_rf) with ||w_rf|| ~ 0.02, so mu_t(p)/phi_t(p)
are nearly-linear in p.  The heavy MLP stacks are evaluated batched over t at
two nodes p in {+1, -1} (node -1 only for the first WIN=256 global steps: rf
saturates to exactly 1.0 by t=14, validated), then a cheap host-side
fixed-point (cumprod) resolves the recurrence.  Device work is data-parallel
over t across 8 NeuronCores (256 own rows + 32 transient rows per core).

Device layout: activations are feature-major ([feat partition, rows free]),
weights bf16 as [K,M] stationary tiles streamed from a host-packed DRAM image,
fp32 PSUM accumulation.  LayerNorm stats (mean / E[x^2]) via ones-vector
matmuls on the PE, broadcast back through a K=1 matmul, applied with two DVE
tensor-tensor ops.  Biases ride free in ScalarE activation bias operands;
LN gain/bias of stack-final blocks are folded into downstream weights on host.
"""

import numpy as np
import ml_dtypes

M = 1024
T = 2048
NS = 5
DE = 64
NB = 3
NCORES = 8
RPC = T // NCORES            # own rows per core (256)
EPS = 1e-5
NI = 6                       # host fixed-point iterations

BF = ml_dtypes.bfloat16

TRACE = False                # test.py sets kernel.TRACE = True for profiling
LAST_RESULTS = None          # stashed BassKernelResults for test.py


def _bf(x):
    return np.ascontiguousarray(np.asarray(x, np.float32).astype(BF))


def _tile_mat(w):
    """[1024,1024] -> [128, 8*8*128] bf16 row-image: row p holds, for kt, of,
    W[kt*128+p, of*128:(of+1)*128] at free offset (kt*8+of)*128."""
    w = np.asarray(w, np.float32)
    return _bf(w.reshape(8, 128, 8, 128).transpose(1, 0, 2, 3).reshape(128, 8192))


class _Layout:
    """Free-dim element offsets inside the misc block of wpack (bf16)."""

    def __init__(self, cfg):
        self.cfg = cfg
        off = 0

        def take(n):
            nonlocal off
            o = off
            off += n
            return o

        self.peff = take(M)             # [5, 1024] on partitions 0:5
        self.descw = take(DE)           # [6, 64] on partitions 0:6
        self.ident = take(128)          # [128, 128]
        self.ones_col = take(1)         # [128, 1] value 1/1024
        self.ones_row = take(128)       # [1, 128] value 1.0
        self.wde = take(M)              # [64, 8*128]: W_de[64,1024] natural
        self.head = take(16)            # [128, 8*2]: head kt tile at +kt*2
        if cfg["b2sp"]:
            self.b2sp = take(3 * M)     # [1, 3*1024] partition 0
        if cfg["b2bh"]:
            self.b2bh = take(3 * M)
        self.misc_len = off
        self.mats = off                 # 13 big matrices follow
        self.total = off + 13 * 8192

        # smalls (fp32 [128, S]) column offsets
        s = 0

        def stake(n):
            nonlocal s
            o = s
            s += n
            return o

        self.s_spb = stake(8)           # sp_proj_b tiles
        self.s_descb = stake(1)         # desc_b on partitions 0:64
        self.s_b1sp = stake(24)         # 3 blocks x 8
        self.s_b1bh = stake(24)
        self.s_zb = stake(16)           # z bias: [:, 0:8]=node +1, [:, 8:16]=node -1
        if cfg["gbsp"]:
            self.s_gsp = stake(16)      # blocks 0,1: g tiles
            self.s_bsp = stake(16)
        if cfg["gbbh"]:
            self.s_gbh = stake(16)
            self.s_bbh = stake(16)
        self.smalls_len = s


_BUILD_CACHE = {}


def _build_program(cfg):
    """Build (nc, layout) for a given cfg. cfg keys:
    ncol: device column count (own rows + transient rows)
    nt:   transient columns (ncol = RPC + nt)
    b2sp/b2bh: emit fc2-bias row matmuls; gbsp/gbbh: emit inner-LN gain/bias.
    """
    key = tuple(sorted(cfg.items()))
    if key in _BUILD_CACHE:
        return _BUILD_CACHE[key]

    import concourse.bass as bass
    import concourse.bacc as bacc
    import concourse.tile as tile
    import concourse.mybir as mybir
    from contextlib import ExitStack

    lay = _Layout(cfg)
    NCOL = cfg["ncol"]
    NT = cfg["nt"]
    F32 = mybir.dt.float32
    BF16 = mybir.dt.bfloat16
    AF = mybir.ActivationFunctionType

    nc = bacc.Bacc("TRN2", target_bir_lowering=False)

    wpack = nc.dram_tensor("wpack", [128, lay.total], BF16, kind="ExternalInput")
    smalls = nc.dram_tensor("smalls", [128, lay.smalls_len], F32, kind="ExternalInput")
    ratt = nc.dram_tensor("ratt", [NS, NCOL], BF16, kind="ExternalInput")
    dest_ = nc.dram_tensor("dest", [6, NCOL], BF16, kind="ExternalInput")
    out = nc.dram_tensor("out", [2, NCOL], F32, kind="ExternalOutput")
    dbg_stage = cfg.get("dbg")
    dbg = None
    if dbg_stage:
        dbg = nc.dram_tensor("dbg", [128, 8, NCOL], BF16, kind="ExternalOutput")

    with tile.TileContext(nc) as tc, ExitStack() as ctx:
        const = ctx.enter_context(tc.tile_pool(name="const", bufs=1))
        wpool = ctx.enter_context(tc.tile_pool(name="wpool", bufs=7))
        xpool = ctx.enter_context(tc.tile_pool(name="xpool", bufs=2))
        hpool = ctx.enter_context(tc.tile_pool(name="hpool", bufs=3))
        spool = ctx.enter_context(tc.tile_pool(name="spool", bufs=4))
        bpool = ctx.enter_context(tc.tile_pool(name="bpool", bufs=4))
        tpool = ctx.enter_context(tc.tile_pool(name="tpool", bufs=12))
        rpool = ctx.enter_context(tc.tile_pool(name="rpool", bufs=10))
        zpool = ctx.enter_context(tc.tile_pool(name="zpool", bufs=1))
        pmm = ctx.enter_context(tc.tile_pool(name="pmm", bufs=4, space="PSUM"))
        prow = ctx.enter_context(tc.tile_pool(name="prow", bufs=2, space="PSUM"))
        pbc = ctx.enter_context(tc.tile_pool(name="pbc", bufs=1, space="PSUM"))

        # ---- constants / small inputs ----
        misc = const.tile([128, lay.misc_len], BF16, tag="misc")
        nc.sync.dma_start(out=misc, in_=wpack[:, 0:lay.misc_len])
        sm = const.tile([128, lay.smalls_len], F32, tag="sm")
        nc.sync.dma_start(out=sm, in_=smalls[:, :])
        rt = const.tile([NS, NCOL], BF16, tag="rt")
        nc.sync.dma_start(out=rt, in_=ratt[:, :])
        dt_ = const.tile([6, NCOL], BF16, tag="dt")
        nc.sync.dma_start(out=dt_, in_=dest_[:, :])

        ident = misc[:, lay.ident:lay.ident + 128]
        ones_col = misc[:, lay.ones_col:lay.ones_col + 1]
        ones_row = misc[0:1, lay.ones_row:lay.ones_row + 128]
        eps_t = const.tile([1, 1], F32, tag="eps")
        nc.vector.memset(eps_t, EPS)
        # ACT instructions encode a single sync-wait; touch the sm DMA once on
        # ScalarE so later ACT bias reads never add a second (DMA) wait.
        warm = const.tile([1, 1], F32, tag="warm")
        nc.scalar.copy(warm, sm[0:1, 0:1])

        def load_mat(i):
            w = wpool.tile([128, 8192], BF16, tag="wmat")
            o = lay.mats + i * 8192
            nc.sync.dma_start(out=w, in_=wpack[:, o:o + 8192])
            return w

        def wt(w, kt, of):
            o = (kt * 8 + of) * 128
            return w[:, o:o + 128]

        def resblock(x, w1, w2, b1_col0, b2_off, gb, n_im, blk=""):
            """x: bf16 tile [128, 8, NCOL] -> returns next x (post-LN)."""
            # fc1 + gelu(+b1)
            h = hpool.tile([128, 8, NCOL], BF16, tag="h")
            for of in range(8):
                ph = pmm.tile([128, NCOL], F32, tag="pmm")
                for kt in range(8):
                    nc.tensor.matmul(ph, lhsT=wt(w1, kt, of), rhs=x[:, kt, :],
                                     start=(kt == 0), stop=(kt == 7))
                nc.scalar.activation(h[:, of, :], ph, AF.Gelu,
                                     bias=sm[:, b1_col0 + of:b1_col0 + of + 1],
                                     scale=1.0)
            dbg_dump(blk + "h", h)
            # fc2 (+b2 row) + residual; copy to sbuf; x^2; stats
            s = spool.tile([128, 8, NCOL], BF16, tag="s")
            pm = prow.tile([1, NCOL], F32, tag="prow")
            pe = prow.tile([1, NCOL], F32, tag="prow")
            for of in range(8):
                ps = pmm.tile([128, NCOL], F32, tag="pmm")
                for kt in range(8):
                    nc.tensor.matmul(ps, lhsT=wt(w2, kt, of), rhs=h[:, kt, :],
                                     start=(kt == 0), stop=False)
                if b2_off is not None:
                    nc.tensor.matmul(ps, lhsT=misc[0:1, b2_off + of * 128:
                                                    b2_off + of * 128 + 128],
                                     rhs=ones_n, start=False, stop=False)
                nc.tensor.matmul(ps, lhsT=ident, rhs=x[:, of, :],
                                 start=False, stop=True)
                nc.scalar.activation(s[:, of, :], ps, AF.Copy)
                x2 = tpool.tile([128, NCOL], BF16, tag="x2")
                nc.vector.tensor_mul(x2, s[:, of, :], s[:, of, :])
                nc.tensor.matmul(pm, lhsT=ones_col, rhs=s[:, of, :],
                                 start=(of == 0), stop=(of == 7))
                nc.tensor.matmul(pe, lhsT=ones_col, rhs=x2,
                                 start=(of == 0), stop=(of == 7))
            dbg_dump(blk + "s", s)
            # rows: m, inv = 1/sqrt(E - m^2 + eps)
            rowb = rpool.tile([1, 2, NCOL], BF16, tag="rowb")
            nc.vector.tensor_copy(rowb[:, 0, :], pm)
            msq = rpool.tile([1, NCOL], F32, tag="rtmp")
            nc.vector.tensor_mul(msq, rowb[:, 0, :], rowb[:, 0, :])
            var = rpool.tile([1, NCOL], F32, tag="rtmp")
            nc.vector.tensor_sub(var, pe, msq)
            sd = rpool.tile([1, NCOL], F32, tag="rtmp")
            nc.scalar.activation(sd, var, AF.Sqrt, bias=eps_t, scale=1.0)
            with nc.allow_low_precision(reason="inv feeds bf16 multiply"):
                nc.vector.reciprocal(rowb[:, 1, :], sd)
            # broadcast [m | inv] to 128 partitions
            pb = pbc.tile([128, 2 * NCOL], F32, tag="pbc")
            flat = rowb.rearrange("p a b -> p (a b)")
            for c0 in range(0, 2 * NCOL, 512):
                c1 = min(c0 + 512, 2 * NCOL)
                nc.tensor.matmul(pb[:, c0:c1], lhsT=ones_row,
                                 rhs=flat[:, c0:c1], start=True, stop=True)
            bsb = bpool.tile([128, 2 * NCOL], BF16, tag="bsb")
            nc.scalar.activation(bsb, pb, AF.Copy)
            # apply (s - m) * inv (+ optional g,b)
            xn = xpool.tile([128, 8, NCOL], BF16, tag="x")
            for of in range(8):
                tt = tpool.tile([128, NCOL], BF16, tag="x2")
                nc.vector.tensor_sub(tt, s[:, of, :], bsb[:, 0:NCOL])
                nc.vector.tensor_mul(xn[:, of, :], tt, bsb[:, NCOL:2 * NCOL])
                if gb is not None:
                    g_c, b_c = gb
                    nc.scalar.activation(
                        xn[:, of, :], xn[:, of, :], AF.Identity,
                        bias=sm[:, b_c + n_im * 8 + of:b_c + n_im * 8 + of + 1],
                        scale=sm[:, g_c + n_im * 8 + of:g_c + n_im * 8 + of + 1])
            return xn

        ones_n = None
        if cfg["b2sp"] or cfg["b2bh"]:
            ones_n = const.tile([1, NCOL], BF16, tag="ones_n")
            nc.vector.memset(ones_n, 1.0)

        def dbg_dump(name, t):
            if dbg_stage == name:
                nc.sync.dma_start(out=dbg[:, :, :], in_=t)

        # ---- solvent projection: sp_pre = gelu(P_eff.T-form @ ratiosT + b) ----
        x = xpool.tile([128, 8, NCOL], BF16, tag="x")
        for of in range(8):
            pp = pmm.tile([128, NCOL], F32, tag="pmm")
            nc.tensor.matmul(pp, lhsT=misc[0:NS, lay.peff + of * 128:
                                           lay.peff + of * 128 + 128],
                             rhs=rt, start=True, stop=True)
            nc.scalar.activation(x[:, of, :], pp, AF.Gelu,
                                 bias=sm[:, lay.s_spb + of:lay.s_spb + of + 1],
                                 scale=1.0)
        dbg_dump("sppre", x)

        # ---- sp residual blocks ----
        for i in range(NB):
            w1 = load_mat(2 * i)
            w2 = load_mat(2 * i + 1)
            b2o = (lay.b2sp + i * M) if cfg["b2sp"] else None
            gb = ((lay.s_gsp, lay.s_bsp) if (cfg["gbsp"] and i < 2) else None)
            x = resblock(x, w1, w2, lay.s_b1sp + i * 8, b2o, gb, i, blk=f"sp{i + 1}")
            dbg_dump(f"sp{i + 1}", x)

        # ---- desc embedding ----
        pd = prow.tile([DE, NCOL], F32, tag="prow")
        nc.tensor.matmul(pd, lhsT=misc[0:6, lay.descw:lay.descw + DE],
                         rhs=dt_, start=True, stop=True)
        demb = const.tile([DE, NCOL], BF16, tag="demb")
        nc.scalar.activation(demb, pd, AF.Gelu,
                             bias=sm[0:DE, lay.s_descb:lay.s_descb + 1], scale=1.0)

        # ---- base = W_sp' @ sp3 + W_de @ demb (psum), z = gelu(base + zbias_k) ----
        wsp = load_mat(6)
        z = zpool.tile([128, 8, NCOL], BF16, tag="z")
        for of in range(8):
            pbase = pmm.tile([128, NCOL], F32, tag="pmm")
            for kt in range(8):
                nc.tensor.matmul(pbase, lhsT=wt(wsp, kt, of), rhs=x[:, kt, :],
                                 start=(kt == 0), stop=False)
            nc.tensor.matmul(pbase, lhsT=misc[0:DE, lay.wde + of * 128:
                                              lay.wde + of * 128 + 128],
                             rhs=demb, start=False, stop=True)
            # own rows at node +1, transient rows at node -1
            nc.scalar.activation(z[:, of, 0:RPC], pbase[:, 0:RPC], AF.Gelu,
                                 bias=sm[:, lay.s_zb + of:lay.s_zb + of + 1],
                                 scale=1.0)
            nc.scalar.activation(z[:, of, RPC:NCOL], pbase[:, RPC:NCOL], AF.Gelu,
                                 bias=sm[:, lay.s_zb + 8 + of:lay.s_zb + 8 + of + 1],
                                 scale=1.0)

        # ---- beta head residual blocks ----
        x = z
        dbg_dump("z", x)
        for i in range(NB):
            w1 = load_mat(7 + 2 * i)
            w2 = load_mat(8 + 2 * i)
            b2o = (lay.b2bh + i * M) if cfg["b2bh"] else None
            gb = ((lay.s_gbh, lay.s_bbh) if (cfg["gbbh"] and i < 2) else None)
            x = resblock(x, w1, w2, lay.s_b1bh + i * 8, b2o, gb, i, blk=f"bh{i + 1}")
            dbg_dump(f"bh{i + 1}", x)

        # ---- head: [2, NCOL] logits ----
        po = prow.tile([2, NCOL], F32, tag="prow")
        for kt in range(8):
            nc.tensor.matmul(po, lhsT=misc[:, lay.head + kt * 2:lay.head + kt * 2 + 2],
                             rhs=x[:, kt, :], start=(kt == 0), stop=(kt == 7))
        osb = const.tile([2, NCOL], F32, tag="osb")
        nc.scalar.activation(osb, po, AF.Copy)
        nc.sync.dma_start(out=out[:, :], in_=osb)

    nc.compile()
    _BUILD_CACHE[key] = (nc, lay)
    return nc, lay


def kernel(**inputs):
    global LAST_RESULTS
    f = lambda k: np.asarray(inputs[k], np.float32)
    solv, desc = f("solvent_seq"), f("desc_seq")
    molv, sv = f("mol_vec"), f("solvent_vecs")
    bm = np.asarray(inputs["boundary_mask"]).astype(bool)

    sp_ln_g, sp_ln_b = f("sp_ln_g"), f("sp_ln_b")
    bh_ln_g, bh_ln_b = f("bh_ln_g"), f("bh_ln_b")
    sp_fc2_b, bh_fc2_b = f("sp_fc2_b"), f("bh_fc2_b")

    any_bound = bool(bm.any())
    nt = RPC if any_bound else 32          # transient rows per core
    win = nt * NCORES                      # global transient window
    cfg = {
        "ncol": RPC + nt,
        "nt": nt,
        "b2sp": not np.allclose(sp_fc2_b, 0.0),
        "b2bh": not np.allclose(bh_fc2_b, 0.0),
        "gbsp": not (np.allclose(sp_ln_g[:2], 1.0) and np.allclose(sp_ln_b[:2], 0.0)),
        "gbbh": not (np.allclose(bh_ln_g[:2], 1.0) and np.allclose(bh_ln_b[:2], 0.0)),
    }
    NCOL = cfg["ncol"]

    # ---------- host precompute / weight folding ----------
    Wp = f("sp_proj_w").reshape(NS, M, M)
    P_eff = np.stack([sv[s] @ Wp[s] for s in range(NS)]).astype(np.float32)  # [5, M]

    bh_proj_w = f("bh_proj_w")
    W_mol, W_sp = bh_proj_w[:M], bh_proj_w[M:2 * M]
    W_de, w_rf = bh_proj_w[2 * M:2 * M + DE], bh_proj_w[2 * M + DE]
    mol_const = molv @ W_mol

    # fold sp final LN (block 2): sp3 = g*n + b -> n @ (g*W_sp), b@W_sp into bias
    W_sp_f = sp_ln_g[2][:, None] * W_sp
    zb_extra = sp_ln_b[2] @ W_sp
    zb_core = mol_const + f("bh_proj_b") + zb_extra
    zb_p1 = (zb_core + 1.0 * w_rf).astype(np.float32)
    zb_m1 = (zb_core - 1.0 * w_rf).astype(np.float32)

    # fold bh final LN into head: logits = n @ (g*hw) + (b@hw + head_b)
    hw = np.stack([f("mu_w"), f("phi_w")], axis=1)       # [M, 2]
    hw_f = bh_ln_g[2][:, None] * hw
    logit_bias = bh_ln_b[2] @ hw + np.array([f("mu_b")[0], f("phi_b")[0]], np.float32)

    lay = _Layout(cfg)

    # ---------- wpack ----------
    wpack = np.zeros((128, lay.total), BF)
    mi = lay
    wpack[0:NS, mi.peff:mi.peff + M] = _bf(P_eff)
    wpack[0:6, mi.descw:mi.descw + DE] = _bf(f("desc_w"))
    wpack[:, mi.ident:mi.ident + 128] = _bf(np.eye(128, dtype=np.float32))
    wpack[:, mi.ones_col:mi.ones_col + 1] = _bf(np.full((128, 1), 1.0 / M))
    wpack[0:1, mi.ones_row:mi.ones_row + 128] = _bf(np.ones((1, 128)))
    wpack[0:DE, mi.wde:mi.wde + M] = _bf(W_de)           # [64, 1024] natural
    # head tiles: [128, 8*2]
    hh = hw_f.reshape(8, 128, 2).transpose(1, 0, 2).reshape(128, 16)
    wpack[:, mi.head:mi.head + 16] = _bf(hh)
    if cfg["b2sp"]:
        wpack[0:1, mi.b2sp:mi.b2sp + 3 * M] = _bf(sp_fc2_b.reshape(1, 3 * M))
    if cfg["b2bh"]:
        wpack[0:1, mi.b2bh:mi.b2bh + 3 * M] = _bf(bh_fc2_b.reshape(1, 3 * M))
    mats = [f("sp_fc1_w")[0], f("sp_fc2_w")[0],
            f("sp_fc1_w")[1], f("sp_fc2_w")[1],
            f("sp_fc1_w")[2], f("sp_fc2_w")[2],
            W_sp_f,
            f("bh_fc1_w")[0], f("bh_fc2_w")[0],
            f("bh_fc1_w")[1], f("bh_fc2_w")[1],
            f("bh_fc1_w")[2], f("bh_fc2_w")[2]]
    for i, w in enumerate(mats):
        o = lay.mats + i * 8192
        wpack[:, o:o + 8192] = _tile_mat(w)

    # ---------- smalls ----------
    sm = np.zeros((128, lay.smalls_len), np.float32)
    sm[:, lay.s_spb:lay.s_spb + 8] = f("sp_proj_b").reshape(8, 128).T
    sm[0:DE, lay.s_descb] = f("desc_b")
    sm[:, lay.s_b1sp:lay.s_b1sp + 24] = f("sp_fc1_b").reshape(3 * 8, 128).T
    sm[:, lay.s_b1bh:lay.s_b1bh + 24] = f("bh_fc1_b").reshape(3 * 8, 128).T
    sm[:, lay.s_zb:lay.s_zb + 8] = zb_p1.reshape(8, 128).T
    sm[:, lay.s_zb + 8:lay.s_zb + 16] = zb_m1.reshape(8, 128).T
    if cfg["gbsp"]:
        sm[:, lay.s_gsp:lay.s_gsp + 16] = sp_ln_g[:2].reshape(16, 128).T
        sm[:, lay.s_bsp:lay.s_bsp + 16] = sp_ln_b[:2].reshape(16, 128).T
    if cfg["gbbh"]:
        sm[:, lay.s_gbh:lay.s_gbh + 16] = bh_ln_g[:2].reshape(16, 128).T
        sm[:, lay.s_bbh:lay.s_bbh + 16] = bh_ln_b[:2].reshape(16, 128).T

    # ---------- per-core activations ----------
    ratT = _bf(solv.T)                                   # [5, T]
    desT = _bf(desc.T)                                   # [6, T]
    in_maps = []
    for c in range(NCORES):
        own = slice(c * RPC, (c + 1) * RPC)
        tr = slice(c * nt, (c + 1) * nt)
        in_maps.append({
            "wpack": wpack,
            "smalls": sm,
            "ratt": np.concatenate([ratT[:, own], ratT[:, tr]], axis=1),
            "dest": np.concatenate([desT[:, own], desT[:, tr]], axis=1),
        })

    # ---------- run on 8 NeuronCores ----------
    from concourse.bass_utils import run_bass_kernel_spmd
    nc, _ = _build_program(cfg)
    res = run_bass_kernel_spmd(nc, in_maps, core_ids=list(range(NCORES)),
                               trace=TRACE)
    LAST_RESULTS = res

    # ---------- host epilogue ----------
    L1 = np.empty((2, T), np.float32)                    # logits at p=+1
    L0 = np.empty((2, win), np.float32)                  # logits at p=-1 (window)
    for c in range(NCORES):
        o = res.results[c]["out"]
        L1[:, c * RPC:(c + 1) * RPC] = o[:, 0:RPC]
        L0[:, c * nt:(c + 1) * nt] = o[:, RPC:NCOL]
    L1 += logit_bias[:, None]
    L0 += logit_bias[:, None]

    def sigmoid(x):
        return (1.0 / (1.0 + np.exp(-x))).astype(np.float32)

    def softplus(x):
        return (np.log1p(np.exp(-np.abs(x))) + np.maximum(x, 0.0) + 2.0).astype(np.float32)

    mu1, phi1 = sigmoid(L1[0]), softplus(L1[1])
    mu0, phi0 = sigmoid(L0[0]), softplus(L0[1])

    # mu(p) linear in p on the window; == mu1 beyond it
    a_mu = np.concatenate([(mu1[:win] + mu0) * 0.5, mu1[win:]])
    b_mu = np.concatenate([(mu1[:win] - mu0) * 0.5, np.zeros(T - win, np.float32)])
    a_ph = np.concatenate([(phi1[:win] + phi0) * 0.5, phi1[win:]])
    b_ph = np.concatenate([(phi1[:win] - phi0) * 0.5, np.zeros(T - win, np.float32)])

    if not any_bound:
        p = np.ones(T, np.float32)
        p[0] = -1.0
        for _ in range(NI):
            mu = (a_mu + b_mu * p).astype(np.float32)
            with np.errstate(under="ignore"):
                rf = (1.0 - np.exp(np.cumsum(np.log1p(-mu), dtype=np.float32)))
            p = np.concatenate(([np.float32(-1.0)], rf[:-1].astype(np.float32)))
        mu = (a_mu + b_mu * p).astype(np.float32)
        phi = (a_ph + b_ph * p).astype(np.float32)
        with np.errstate(under="ignore"):
            rf = (1.0 - np.exp(np.cumsum(np.log1p(-mu), dtype=np.float32))).astype(np.float32)
    else:
        rf = np.empty(T, np.float32)
        mu = np.empty(T, np.float32)
        phi = np.empty(T, np.float32)
        prev = np.float32(-1.0)
        for t in range(T):
            mt = np.float32(a_mu[t] + b_mu[t] * prev)
            pt = np.float32(a_ph[t] + b_ph[t] * prev)
            r = mt if (bm[t] or prev < 0) else np.float32(prev + mt * (1.0 - prev))
            rf[t], mu[t], phi[t] = r, mt, pt
            prev = r

    return np.stack([rf, mu, phi]).astype(np.float32)
